# revision 4
# baseline (speedup 1.0000x reference)
"""Trainium2 Bass kernel for nn_CamFusionModule (epipolar max-sampling fusion).

Design (host-scheduled windowed gather):

Host (bit-exact jax-CPU camera math, as the reference):
  * per (pair, sweep, t) rounded sample indices for all 4096 pixels.
  * pixels sorted per pair by epipolar-line parameter -> 128-px blocks
    whose index values cluster into narrow y-windows.
  * work items (pair, sweep, t-pair, px-block, y-window of 16): for each,
    a one-hot fp8 mask [32=(2 parity x 16 y_off), 128 px] and an fp16
    table [32, 32=(2 parity x 16 ch)] holding the heatmap samples.
  * items are grouped by (pair, px-block), padded to 16 (= one PSUM
    bank), bin-packed across the 8 cores, and packed into DMA strips.

Device (identical SPMD program on 8 cores; only the data differs):
  stream strips -> per item one K=32 matmul (mask stationary fp8,
  table moving fp16) gathering 2x16-channel samples for 128 px into a
  PSUM bank slot; after 16 items, max-reduce the bank [128, 512] ->
  [128, 16] (DVE tensor_reduce / GPSIMD max tree, 2:1 split); batch
  results stream to DRAM.

Host combines per-group batches (max), unpermutes pixels, reassembles
[4, 3, 16, 64, 64]. Zero padding is exact: heatmaps are non-negative
and the reference floors partially-OOB lines at 0.
"""

import numpy as np
import ml_dtypes

NVIEW = 4
B, C, H, W = 1, 16, 64, 64
HW = H * W
NPAIR = 12
NCORE = 8
PXB = 128            # pixels per matmul block (M)
WIN = 16             # y-window height
BANK = 16            # items per PSUM bank / drain batch
SPF = 64             # items per strip per partition slot
SLOTS = 3            # partition slots per strip (base 0/32/64)
SITEMS = SPF * SLOTS  # 192 items per strip

_PAIRS = [(c, o) for c in range(NVIEW) for o in range(NVIEW) if o != c]
_F8 = ml_dtypes.float8_e4m3


def _line_coords(affine_trans, cam_Intri, cam_R, cam_T, inv_affine_trans):
    """Reference-exact rounded sample indices.
    Returns idx [12, 2, 64, 4096] float32 where idx[p, 0, t, px] is the
    x-sweep row index (sample hm[o, ch, idx, t]) and idx[p, 1, t, px] the
    y-sweep column index (sample hm[o, ch, t, idx]); invalid -> -1."""
    import jax
    import jax.numpy as jnp
    cpu = jax.devices("cpu")[0]
    with jax.default_device(cpu):
        V = NVIEW
        h, w = H, W
        BIG = 1.0e9
        yy, xx = jnp.meshgrid(jnp.arange(h, dtype=jnp.float32),
                              jnp.arange(w, dtype=jnp.float32), indexing='ij')
        onehm = jnp.stack([xx.reshape(-1), yy.reshape(-1),
                           jnp.ones(HW, jnp.float32)], 0)
        K = jnp.asarray(cam_Intri).reshape(B, V, 3, 3)
        R = jnp.asarray(cam_R).reshape(B, V, 3, 3)
        T = jnp.asarray(cam_T).reshape(B, V, 3, 1)
        Aff = jnp.asarray(affine_trans).reshape(B, V, 3, 3)
        invAff = jnp.asarray(inv_affine_trans).reshape(B, V, 3, 3)
        invK = jnp.linalg.inv(K)
        ray = jnp.einsum('bvij,bvjk,kp->bvip', invK, invAff, onehm)
        deps = jnp.array([1000.0, 5000.0], jnp.float32).reshape(2, 1, 1, 1, 1)
        xg = jnp.einsum('bvji,dbvjp->dbvip', R, deps * ray[None]) + T[None]
        xcam = jnp.einsum('boij,dbcojp->dbcoip', R, xg[:, :, :, None] - T[:, None])
        xnorm = xcam / xcam[:, :, :, :, 2:3]
        M = jnp.einsum('bvij,bvjk->bvik', Aff, K)
        uv = jnp.einsum('boij,dbcojp->dbcoip', M, xnorm)
        oth = np.array([[o for o in range(V) if o != c] for c in range(V)])
        uv = uv[:, :, jnp.arange(V)[:, None], oth]
        x0, y0 = uv[0, ..., 0, :], uv[0, ..., 1, :]
        x1, y1 = uv[1, ..., 0, :], uv[1, ..., 1, :]
        kk = (y1 - y0) / (x1 - x0)
        xs = jnp.arange(w, dtype=jnp.float32)
        ysw = kk[..., None] * (xs - x0[..., None]) + y0[..., None]
        ysh = jnp.arange(h, dtype=jnp.float32)
        xsh = (ysh - y0[..., None]) / kk[..., None] + x0[..., None]

        def _round_chain(v):
            v = jnp.where(jnp.isfinite(v), v, jnp.float32(BIG))
            g = v / jnp.float32((W - 1) / 2.0) - 1.0
            return jnp.round((g + 1.0) * 0.5 * (W - 1))

        iy = np.asarray(_round_chain(ysw), np.float32)  # (B,V,V-1,HW,w)
        ix = np.asarray(_round_chain(xsh), np.float32)
    iy = iy.reshape(NPAIR, HW, W).transpose(0, 2, 1)    # [12, t, px]
    ix = ix.reshape(NPAIR, HW, H).transpose(0, 2, 1)
    idx = np.stack([iy, ix], axis=1)                    # [12, 2, 64, 4096]
    raw = np.clip(idx, -3000.0, 3000.0).astype(np.float32)
    valid = (idx >= 0) & (idx <= 63)
    idx = np.where(valid, idx, -1.0).astype(np.float32)
    return idx, raw


def _schedule(idx, raw):
    """Host scheduler. Returns per-pair perms and per-core schedules.

    Each schedule is a list of batches; each batch is (group_key, items)
    with exactly BANK items (None-padded); item = (p, s, g, blk, wb)."""
    perms = np.empty((NPAIR, HW), np.int64)
    groups = {}   # (p, blk) -> list of items
    for p in range(NPAIR):
        key1 = raw[p, 0, 32]
        key2 = raw[p, 0, 48] - raw[p, 0, 16]
        perm = np.lexsort((key2, key1))
        perms[p] = perm
        for s in range(2):
            a = idx[p, s][:, perm]                      # [64, 4096]
            for blk in range(HW // PXB):
                sl = a[:, blk * PXB:(blk + 1) * PXB]
                for g in range(W // 2):
                    rows = sl[2 * g:2 * g + 2]
                    vv = rows >= 0
                    if not vv.any():
                        continue
                    vals = rows[vv]
                    lo, hi = int(vals.min()), int(vals.max())
                    for wb in range(lo, hi + 1, WIN):
                        groups.setdefault((p, blk), []).append(
                            (p, s, g, blk, wb))
    # pad each group to a multiple of BANK
    for k, items in groups.items():
        pad = (-len(items)) % BANK
        items.extend([None] * pad)
    # greedy bin-pack groups across cores
    order = sorted(groups, key=lambda k: -len(groups[k]))
    loads = [0] * NCORE
    core_groups = [[] for _ in range(NCORE)]
    for k in order:
        c = int(np.argmin(loads))
        core_groups[c].append(k)
        loads[c] += len(groups[k])

    # pack each core's banks into the drain calendar: period-7 pattern
    # [A B A B A B S]; (A,B) positions form a pair that must hold two
    # banks of the same group; S holds one bank.
    pad_bank = (None, [None] * BANK)
    core_banks = []
    for c in range(NCORE):
        rem = {}
        for k in core_groups[c]:
            items = groups[k]
            rem[k] = [items[b0:b0 + BANK]
                      for b0 in range(0, len(items), BANK)]
        seq = []
        while rem:
            t = _cal(len(seq))
            if t == 'A':
                k = max(rem, key=lambda g: len(rem[g]))
                if len(rem[k]) >= 2:
                    seq.append((k, rem[k].pop(0)))
                    seq.append((k, rem[k].pop(0)))
                else:
                    seq.append((k, rem[k].pop(0)))
                    seq.append((k, [None] * BANK))
                if not rem[k]:
                    del rem[k]
            else:  # S
                odd = [g for g in rem if len(rem[g]) % 2 == 1]
                k = min(odd, key=lambda g: len(rem[g])) if odd else \
                    max(rem, key=lambda g: len(rem[g]))
                seq.append((k, rem[k].pop(0)))
                if not rem[k]:
                    del rem[k]
        core_banks.append(seq)
    nbank = max(len(s) for s in core_banks)
    # align all cores to nbank with pad banks (calendar-safe: pads can
    # sit at any position; a pad pair or pad single drains zeros)
    while _cal(nbank) == 'B':
        nbank += 1
    for s in core_banks:
        while len(s) < nbank:
            s.append(pad_bank)
    nitem = nbank * BANK
    nstrip = -(-nitem // SITEMS)
    return perms, core_banks, nstrip, nitem


_CAL13 = ('S', 'A', 'B', 'A', 'B', 'A', 'B',
          'S', 'A', 'B', 'A', 'B', 'S')


def _cal(b):
    """Drain calendar: bank position -> 'A' (pair first), 'B' (pair
    second, drains both), 'S' (single direct drain). Singles lead each
    period so the DVE can start before the first ACT copies land; an
    extra run of singles at the very start hides the ACT warm-up."""
    if b < 3:
        return 'S'
    return _CAL13[(b - 3) % 13]


def _ndrains(nbank):
    return sum(1 for b in range(nbank) if _cal(b) in ('B', 'S'))


def _pack_core(sched, idx, perms, hmp_x, hmp_y, nstrip):
    """Build one core's strip arrays from its schedule."""
    msk = np.zeros((nstrip, 96, SPF * PXB), _F8)
    tab = np.zeros((nstrip, 96, SPF * 32), np.float16)
    yoff = np.arange(WIN, dtype=np.float32)
    it = 0
    for (gk, items) in sched:
        for item in items:
            st, loc = divmod(it, SITEMS)
            j, f = divmod(loc, SPF)
            it += 1
            if item is None:
                continue
            p, s, g, blk, wb = item
            o = _PAIRS[p][1]
            px = perms[p][blk * PXB:(blk + 1) * PXB]
            rows = idx[p, s][2 * g:2 * g + 2][:, px]      # [2, 128]
            m = (rows[:, None, :] == (wb + yoff)[None, :, None])
            msk[st, 32 * j:32 * j + 32, f * PXB:(f + 1) * PXB] = \
                m.reshape(32, PXB).astype(_F8)
            # table [32, 32]: row par*16+y, col par*16+ch (block diagonal)
            t32 = np.zeros((32, 32), np.float16)
            for par in range(2):
                t = 2 * g + par
                if s == 0:
                    blkv = hmp_x[o, :, wb:wb + WIN, t]    # [ch, y]
                else:
                    blkv = hmp_y[o, :, t, wb:wb + WIN]    # [ch, x]
                t32[par * 16:par * 16 + 16, par::2] = blkv.T
            tab[st, 32 * j:32 * j + 32, f * 32:(f + 1) * 32] = t32
    return msk, tab


_COMPILED = {}


DR_MOD = 10       # of every DR_MOD banks, DR_ASSIST drain via ACT copy
DR_ASSIST = 9


def _build_program(nstrip, nitem):
    import concourse.bacc as bacc
    import concourse.mybir as mybir
    import concourse.tile as tile
    from contextlib import ExitStack

    dt = mybir.dt
    ops = mybir.AluOpType
    nb = nitem // BANK
    nd = _ndrains(nb)

    nc = bacc.Bacc("TRN2", target_bir_lowering=False, debug=False,
                   num_devices=NCORE)
    msk_d = nc.dram_tensor("msk", [nstrip, 96, SPF * PXB], dt.float8e4,
                           kind="ExternalInput")
    tab_d = nc.dram_tensor("tab", [nstrip, 96, SPF * 32], dt.float16,
                           kind="ExternalInput")
    out_d = nc.dram_tensor("out", [128, nd * 16], dt.float16,
                           kind="ExternalOutput")

    with tile.TileContext(nc) as tc:
        with ExitStack() as ctx:
            spool = ctx.enter_context(tc.tile_pool(name="strips", bufs=7))
            ppool = ctx.enter_context(tc.tile_pool(name="banks", bufs=8,
                                                   space="PSUM"))
            apool = ctx.enter_context(tc.tile_pool(name="accs", bufs=4))
            dpool = ctx.enter_context(tc.tile_pool(name="scr", bufs=6))

            acc = None
            ps = None
            scrA = None
            dr = 0          # drain event counter
            for st in range(nstrip):
                mk = spool.tile([96, SPF * PXB], dt.float8e4, tag="mk")
                tb = spool.tile([96, SPF * 32], dt.float16, tag="tb")
                if st == 0:
                    # split first strip: unlock banks in consumption order
                    nc.sync.dma_start(mk[0:32, 0:16 * PXB],
                                      msk_d.ap()[0, 0:32, 0:16 * PXB])
                    nc.sync.dma_start(tb[0:32, 0:16 * 32],
                                      tab_d.ap()[0, 0:32, 0:16 * 32])
                    nc.sync.dma_start(mk[0:32, 16 * PXB:],
                                      msk_d.ap()[0, 0:32, 16 * PXB:])
                    nc.sync.dma_start(mk[32:96, :], msk_d.ap()[0, 32:96, :])
                    nc.sync.dma_start(tb[0:32, 16 * 32:],
                                      tab_d.ap()[0, 0:32, 16 * 32:])
                    nc.sync.dma_start(tb[32:96, :], tab_d.ap()[0, 32:96, :])
                else:
                    nc.sync.dma_start(mk[:], msk_d.ap()[st])
                    nc.sync.dma_start(tb[:], tab_d.ap()[st])
                for j in range(SLOTS):
                    for f in range(SPF):
                        k = st * SITEMS + j * SPF + f
                        if k >= nitem:
                            break
                        bs = k % BANK
                        if bs == 0:
                            ps = ppool.tile([128, 512], dt.float32, tag="bank")
                        pv = ps[:].rearrange("p (c s q) -> p c s q",
                                             c=16, s=16, q=2)
                        nc.tensor.matmul(
                            pv[:, :, bs, :],
                            mk[32 * j:32 * j + 32, PXB * f:PXB * (f + 1)],
                            tb[32 * j:32 * j + 32, 32 * f:32 * (f + 1)],
                            start=True, stop=True)
                        if bs == BANK - 1:
                            b = k // BANK
                            t = _cal(b)
                            if t == 'A':
                                scrA = dpool.tile([128, 512], dt.float16,
                                                  tag="scrA")
                                nc.scalar.copy(scrA[:], ps[:])
                                continue
                            if dr % 16 == 0:
                                acc = apool.tile([128, 256], dt.float16,
                                                 tag="acc")
                            dst = acc[:, 16 * (dr % 16):16 * (dr % 16) + 16]
                            if t == 'B':
                                scrB = dpool.tile([128, 512], dt.float16,
                                                  tag="scrB")
                                nc.scalar.copy(scrB[:], ps[:])
                                nc.vector.tensor_tensor(
                                    scrA[:], scrA[:], scrB[:], ops.max)
                                v = scrA[:].rearrange("p (c w) -> p c w",
                                                      c=16)
                                # fold the 32-wide runs down to 8 before
                                # the final reduce (fp16 2x DVE folds;
                                # reduce w8 is cheaper than fold4+reduce4)
                                for w in (16, 8):
                                    nc.vector.tensor_tensor(
                                        v[:, :, 0:w], v[:, :, 0:w],
                                        v[:, :, w:2 * w], ops.max)
                                nc.vector.tensor_reduce(
                                    dst, v[:, :, 0:8],
                                    mybir.AxisListType.X, ops.max)
                            else:  # 'S'
                                v = ps[:].rearrange("p (c w) -> p c w", c=16)
                                nc.vector.tensor_reduce(
                                    dst, v, mybir.AxisListType.X, ops.max)
                            dr += 1
                            if dr % 16 == 0 or dr == nd:
                                d0 = 16 * ((dr - 1) // 16)
                                nc.gpsimd.dma_start(
                                    out_d.ap()[:, 16 * d0:16 * dr],
                                    acc[:, 0:16 * (dr - d0)])
    nc.compile()
    return nc


def kernel(heatmaps, affine_trans, cam_Intri, cam_R, cam_T, inv_affine_trans):
    from concourse.bass_utils import run_bass_kernel_spmd

    heatmaps = np.asarray(heatmaps)
    in_dtype = heatmaps.dtype

    idx, raw = _line_coords(affine_trans, cam_Intri, cam_R, cam_T,
                            inv_affine_trans)
    perms, scheds, nstrip, nitem = _schedule(idx, raw)

    hm16 = np.asarray(heatmaps, np.float32).reshape(NVIEW, C, H, W)
    hm16 = hm16.astype(np.float16)
    # zero-pad so y-windows may overhang past 63
    hmp_x = np.zeros((NVIEW, C, H + WIN, W), np.float16)
    hmp_x[:, :, :H, :] = hm16
    hmp_y = np.zeros((NVIEW, C, H, W + WIN), np.float16)
    hmp_y[:, :, :, :W] = hm16

    in_maps = []
    for ci in range(NCORE):
        msk, tab = _pack_core(scheds[ci], idx, perms, hmp_x, hmp_y, nstrip)
        in_maps.append({"msk": msk, "tab": tab})

    key = (nstrip, nitem)
    if key not in _COMPILED:
        _COMPILED[key] = _build_program(nstrip, nitem)
    nc = _COMPILED[key]

    res = run_bass_kernel_spmd(nc, in_maps, list(range(NCORE)))

    out_full = np.zeros((NPAIR, C, HW), np.float32)
    for ci in range(NCORE):
        ob = res.results[ci]["out"].astype(np.float32)
        agg = {}
        dr = 0
        for b, (gk, items) in enumerate(scheds[ci]):
            if _cal(b) == 'A':
                continue
            ev = ob[:, 16 * dr:16 * dr + 16]
            dr += 1
            if gk is None:
                continue
            if gk in agg:
                np.maximum(agg[gk], ev, out=agg[gk])
            else:
                agg[gk] = ev.copy()
        for (p, blk), cur in agg.items():
            px = perms[p][blk * PXB:(blk + 1) * PXB]
            out_full[p][:, px] = cur.T

    out = np.zeros((NVIEW, NVIEW - 1, C, H, W), np.float32)
    for p, (c, o) in enumerate(_PAIRS):
        slot = [v for v in range(NVIEW) if v != c].index(o)
        out[c, slot] = out_full[p].reshape(C, H, W)
    return out.astype(in_dtype, copy=False)


# revision 5
# speedup vs baseline: 1.0500x; 1.0500x over previous
"""Trainium2 Bass kernel for nn_CamFusionModule (epipolar max-sampling fusion).

Design (host-scheduled windowed gather):

Host (bit-exact jax-CPU camera math, as the reference):
  * per (pair, sweep, t) rounded sample indices for all 4096 pixels.
  * pixels sorted per pair by epipolar-line parameter -> 128-px blocks
    whose index values cluster into narrow y-windows.
  * work items (pair, sweep, t-pair, px-block, y-window of 16): for each,
    a one-hot fp8 mask [32=(2 parity x 16 y_off), 128 px] and an fp16
    table [32, 32=(2 parity x 16 ch)] holding the heatmap samples.
  * items are grouped by (pair, px-block), padded to 16 (= one PSUM
    bank), bin-packed across the 8 cores, and packed into DMA strips.

Device (identical SPMD program on 8 cores; only the data differs):
  stream strips -> per item one K=32 matmul (mask stationary fp8,
  table moving fp16) gathering 2x16-channel samples for 128 px into a
  PSUM bank slot; after 16 items, max-reduce the bank [128, 512] ->
  [128, 16] (DVE tensor_reduce / GPSIMD max tree, 2:1 split); batch
  results stream to DRAM.

Host combines per-group batches (max), unpermutes pixels, reassembles
[4, 3, 16, 64, 64]. Zero padding is exact: heatmaps are non-negative
and the reference floors partially-OOB lines at 0.
"""

import numpy as np
import ml_dtypes

NVIEW = 4
B, C, H, W = 1, 16, 64, 64
HW = H * W
NPAIR = 12
NCORE = 8
PXB = 128            # pixels per matmul block (M)
WIN = 16             # y-window height
BANK = 16            # items per PSUM bank / drain batch
SPF = 64             # items per strip per partition slot
SLOTS = 3            # partition slots per strip (base 0/32/64)
SITEMS = SPF * SLOTS  # 192 items per strip

_PAIRS = [(c, o) for c in range(NVIEW) for o in range(NVIEW) if o != c]
_F8 = ml_dtypes.float8_e4m3


def _line_coords(affine_trans, cam_Intri, cam_R, cam_T, inv_affine_trans):
    """Reference-exact rounded sample indices.
    Returns idx [12, 2, 64, 4096] float32 where idx[p, 0, t, px] is the
    x-sweep row index (sample hm[o, ch, idx, t]) and idx[p, 1, t, px] the
    y-sweep column index (sample hm[o, ch, t, idx]); invalid -> -1."""
    import jax
    import jax.numpy as jnp
    cpu = jax.devices("cpu")[0]
    with jax.default_device(cpu):
        V = NVIEW
        h, w = H, W
        BIG = 1.0e9
        yy, xx = jnp.meshgrid(jnp.arange(h, dtype=jnp.float32),
                              jnp.arange(w, dtype=jnp.float32), indexing='ij')
        onehm = jnp.stack([xx.reshape(-1), yy.reshape(-1),
                           jnp.ones(HW, jnp.float32)], 0)
        K = jnp.asarray(cam_Intri).reshape(B, V, 3, 3)
        R = jnp.asarray(cam_R).reshape(B, V, 3, 3)
        T = jnp.asarray(cam_T).reshape(B, V, 3, 1)
        Aff = jnp.asarray(affine_trans).reshape(B, V, 3, 3)
        invAff = jnp.asarray(inv_affine_trans).reshape(B, V, 3, 3)
        invK = jnp.linalg.inv(K)
        ray = jnp.einsum('bvij,bvjk,kp->bvip', invK, invAff, onehm)
        deps = jnp.array([1000.0, 5000.0], jnp.float32).reshape(2, 1, 1, 1, 1)
        xg = jnp.einsum('bvji,dbvjp->dbvip', R, deps * ray[None]) + T[None]
        xcam = jnp.einsum('boij,dbcojp->dbcoip', R, xg[:, :, :, None] - T[:, None])
        xnorm = xcam / xcam[:, :, :, :, 2:3]
        M = jnp.einsum('bvij,bvjk->bvik', Aff, K)
        uv = jnp.einsum('boij,dbcojp->dbcoip', M, xnorm)
        oth = np.array([[o for o in range(V) if o != c] for c in range(V)])
        uv = uv[:, :, jnp.arange(V)[:, None], oth]
        x0, y0 = uv[0, ..., 0, :], uv[0, ..., 1, :]
        x1, y1 = uv[1, ..., 0, :], uv[1, ..., 1, :]
        kk = (y1 - y0) / (x1 - x0)
        xs = jnp.arange(w, dtype=jnp.float32)
        ysw = kk[..., None] * (xs - x0[..., None]) + y0[..., None]
        ysh = jnp.arange(h, dtype=jnp.float32)
        xsh = (ysh - y0[..., None]) / kk[..., None] + x0[..., None]

        def _round_chain(v):
            v = jnp.where(jnp.isfinite(v), v, jnp.float32(BIG))
            g = v / jnp.float32((W - 1) / 2.0) - 1.0
            return jnp.round((g + 1.0) * 0.5 * (W - 1))

        iy = np.asarray(_round_chain(ysw), np.float32)  # (B,V,V-1,HW,w)
        ix = np.asarray(_round_chain(xsh), np.float32)
    iy = iy.reshape(NPAIR, HW, W).transpose(0, 2, 1)    # [12, t, px]
    ix = ix.reshape(NPAIR, HW, H).transpose(0, 2, 1)
    idx = np.stack([iy, ix], axis=1)                    # [12, 2, 64, 4096]
    raw = np.clip(idx, -3000.0, 3000.0).astype(np.float32)
    valid = (idx >= 0) & (idx <= 63)
    idx = np.where(valid, idx, -1.0).astype(np.float32)
    return idx, raw


def _schedule(idx, raw):
    """Host scheduler. Returns per-pair perms and per-core schedules.

    Each schedule is a list of batches; each batch is (group_key, items)
    with exactly BANK items (None-padded); item = (p, s, g, blk, wb)."""
    perms = np.empty((NPAIR, HW), np.int64)
    groups = {}   # (p, blk) -> list of items
    for p in range(NPAIR):
        key1 = raw[p, 0, 32]
        key2 = raw[p, 0, 48] - raw[p, 0, 16]
        perm = np.lexsort((key2, key1))
        perms[p] = perm
        for s in range(2):
            a = idx[p, s][:, perm]                      # [64, 4096]
            for blk in range(HW // PXB):
                sl = a[:, blk * PXB:(blk + 1) * PXB]
                for g in range(W // 2):
                    rows = sl[2 * g:2 * g + 2]
                    vv = rows >= 0
                    if not vv.any():
                        continue
                    vals = rows[vv]
                    lo, hi = int(vals.min()), int(vals.max())
                    for wb in range(lo, hi + 1, WIN):
                        groups.setdefault((p, blk), []).append(
                            (p, s, g, blk, wb))
    # pad each group to a multiple of BANK
    for k, items in groups.items():
        pad = (-len(items)) % BANK
        items.extend([None] * pad)
    # greedy bin-pack groups across cores
    order = sorted(groups, key=lambda k: -len(groups[k]))
    loads = [0] * NCORE
    core_groups = [[] for _ in range(NCORE)]
    for k in order:
        c = int(np.argmin(loads))
        core_groups[c].append(k)
        loads[c] += len(groups[k])

    # pack each core's banks into the drain calendar: period-7 pattern
    # [A B A B A B S]; (A,B) positions form a pair that must hold two
    # banks of the same group; S holds one bank.
    pad_bank = (None, [None] * BANK)
    core_banks = []
    for c in range(NCORE):
        rem = {}
        for k in core_groups[c]:
            items = groups[k]
            rem[k] = [items[b0:b0 + BANK]
                      for b0 in range(0, len(items), BANK)]
        seq = []
        while rem:
            t = _cal(len(seq))
            if t == 'A':
                k = max(rem, key=lambda g: len(rem[g]))
                if len(rem[k]) >= 2:
                    seq.append((k, rem[k].pop(0)))
                    seq.append((k, rem[k].pop(0)))
                else:
                    seq.append((k, rem[k].pop(0)))
                    seq.append((k, [None] * BANK))
                if not rem[k]:
                    del rem[k]
            else:  # S
                odd = [g for g in rem if len(rem[g]) % 2 == 1]
                k = min(odd, key=lambda g: len(rem[g])) if odd else \
                    max(rem, key=lambda g: len(rem[g]))
                seq.append((k, rem[k].pop(0)))
                if not rem[k]:
                    del rem[k]
        core_banks.append(seq)
    nbank = max(len(s) for s in core_banks)
    # align all cores to nbank with pad banks (calendar-safe: pads can
    # sit at any position; a pad pair or pad single drains zeros)
    while _cal(nbank) == 'B':
        nbank += 1
    for s in core_banks:
        while len(s) < nbank:
            s.append(pad_bank)
    nitem = nbank * BANK
    nstrip = -(-nitem // SITEMS)
    return perms, core_banks, nstrip, nitem


_CAL13 = ('S', 'A', 'B', 'A', 'B', 'A', 'B',
          'U', 'A', 'B', 'A', 'B', 'S')


def _cal(b):
    """Drain calendar: bank position -> 'A' (pair first), 'B' (pair
    second, drains both), 'S' (single direct drain). Singles lead each
    period so the DVE can start before the first ACT copies land; an
    extra run of singles at the very start hides the ACT warm-up."""
    if b < 3:
        return 'S'
    return _CAL13[(b - 3) % 13]


def _ndrains(nbank):
    return sum(1 for b in range(nbank) if _cal(b) in ('B', 'S', 'U'))


def _pack_core(sched, idx, perms, hmp_x, hmp_y, nstrip):
    """Build one core's strip arrays from its schedule."""
    msk = np.zeros((nstrip, 96, SPF * PXB), _F8)
    tab = np.zeros((nstrip, 96, SPF * 32), np.float16)
    yoff = np.arange(WIN, dtype=np.float32)
    it = 0
    for (gk, items) in sched:
        for item in items:
            st, loc = divmod(it, SITEMS)
            j, f = divmod(loc, SPF)
            it += 1
            if item is None:
                continue
            p, s, g, blk, wb = item
            o = _PAIRS[p][1]
            px = perms[p][blk * PXB:(blk + 1) * PXB]
            rows = idx[p, s][2 * g:2 * g + 2][:, px]      # [2, 128]
            m = (rows[:, None, :] == (wb + yoff)[None, :, None])
            msk[st, 32 * j:32 * j + 32, f * PXB:(f + 1) * PXB] = \
                m.reshape(32, PXB).astype(_F8)
            # table [32, 32]: row par*16+y, col par*16+ch (block diagonal)
            t32 = np.zeros((32, 32), np.float16)
            for par in range(2):
                t = 2 * g + par
                if s == 0:
                    blkv = hmp_x[o, :, wb:wb + WIN, t]    # [ch, y]
                else:
                    blkv = hmp_y[o, :, t, wb:wb + WIN]    # [ch, x]
                t32[par * 16:par * 16 + 16, par::2] = blkv.T
            tab[st, 32 * j:32 * j + 32, f * 32:(f + 1) * 32] = t32
    return msk, tab


_COMPILED = {}


DR_MOD = 10       # of every DR_MOD banks, DR_ASSIST drain via ACT copy
DR_ASSIST = 9


def _build_program(nstrip, nitem):
    import concourse.bacc as bacc
    import concourse.mybir as mybir
    import concourse.tile as tile
    from contextlib import ExitStack

    dt = mybir.dt
    ops = mybir.AluOpType
    nb = nitem // BANK
    nd = _ndrains(nb)

    nc = bacc.Bacc("TRN2", target_bir_lowering=False, debug=False,
                   num_devices=NCORE)
    msk_d = nc.dram_tensor("msk", [nstrip, 96, SPF * PXB], dt.float8e4,
                           kind="ExternalInput")
    tab_d = nc.dram_tensor("tab", [nstrip, 96, SPF * 32], dt.float16,
                           kind="ExternalInput")
    out_d = nc.dram_tensor("out", [128, nd * 16], dt.float16,
                           kind="ExternalOutput")

    with tile.TileContext(nc) as tc:
        with ExitStack() as ctx:
            spool = ctx.enter_context(tc.tile_pool(name="strips", bufs=7))
            ppool = ctx.enter_context(tc.tile_pool(name="banks", bufs=2,
                                                   space="PSUM"))
            ppool2 = ctx.enter_context(tc.tile_pool(name="pbanks", bufs=3,
                                                    space="PSUM"))
            apool = ctx.enter_context(tc.tile_pool(name="accs", bufs=4))
            dpool = ctx.enter_context(tc.tile_pool(name="scr", bufs=6))

            acc = None
            ps = None
            scrA = None
            dr = 0          # drain event counter
            for st in range(nstrip):
                mk = spool.tile([96, SPF * PXB], dt.float8e4, tag="mk")
                tb = spool.tile([96, SPF * 32], dt.float16, tag="tb")
                if st == 0:
                    # split first strip: unlock banks in consumption order
                    nc.sync.dma_start(mk[0:32, 0:16 * PXB],
                                      msk_d.ap()[0, 0:32, 0:16 * PXB])
                    nc.sync.dma_start(tb[0:32, 0:16 * 32],
                                      tab_d.ap()[0, 0:32, 0:16 * 32])
                    nc.sync.dma_start(mk[0:32, 16 * PXB:],
                                      msk_d.ap()[0, 0:32, 16 * PXB:])
                    nc.sync.dma_start(mk[32:96, :], msk_d.ap()[0, 32:96, :])
                    nc.sync.dma_start(tb[0:32, 16 * 32:],
                                      tab_d.ap()[0, 0:32, 16 * 32:])
                    nc.sync.dma_start(tb[32:96, :], tab_d.ap()[0, 32:96, :])
                else:
                    nc.sync.dma_start(mk[:], msk_d.ap()[st])
                    nc.sync.dma_start(tb[:], tab_d.ap()[st])
                for j in range(SLOTS):
                    for f in range(SPF):
                        k = st * SITEMS + j * SPF + f
                        if k >= nitem:
                            break
                        bs = k % BANK
                        if bs == 0:
                            bt = _cal(k // BANK)
                            if bt == 'A':
                                ps2 = ppool2.tile([128, 1024], dt.float32,
                                                  tag="pair")
                                ps = ps2[:, 0:512]
                            elif bt == 'B':
                                ps = ps2[:, 512:1024]
                            else:
                                pst = ppool.tile([128, 512], dt.float32,
                                                 tag="bank", name="sbank")
                                ps = pst[:]
                        pv = ps.rearrange("p (c s q) -> p c s q",
                                          c=16, s=16, q=2)
                        nc.tensor.matmul(
                            pv[:, :, bs, :],
                            mk[32 * j:32 * j + 32, PXB * f:PXB * (f + 1)],
                            tb[32 * j:32 * j + 32, 32 * f:32 * (f + 1)],
                            start=True, stop=True)
                        if bs == BANK - 1:
                            b = k // BANK
                            t = _cal(b)
                            if t == 'A':
                                continue
                            if dr % 16 == 0:
                                acc = apool.tile([128, 256], dt.float16,
                                                 tag="acc")
                            dst = acc[:, 16 * (dr % 16):16 * (dr % 16) + 16]
                            if t == 'B':
                                # one ACT copy evacuates both banks of
                                # the pair, then fp16 DVE max + folds
                                scr2 = dpool.tile([128, 1024], dt.float16,
                                                  tag="scr2")
                                nc.scalar.copy(scr2[:], ps2[:])
                                h0 = scr2[:, 0:512]
                                nc.vector.tensor_tensor(
                                    h0, h0, scr2[:, 512:1024], ops.max)
                                v = h0.rearrange("p (c w) -> p c w", c=16)
                                for w in (16, 8):
                                    nc.vector.tensor_tensor(
                                        v[:, :, 0:w], v[:, :, 0:w],
                                        v[:, :, w:2 * w], ops.max)
                                nc.vector.tensor_reduce(
                                    dst, v[:, :, 0:8],
                                    mybir.AxisListType.X, ops.max)
                            elif t == 'U':
                                scr = dpool.tile([128, 512], dt.float16,
                                                 tag="scrU")
                                nc.scalar.copy(scr[:], ps)
                                v = scr[:].rearrange("p (c w) -> p c w", c=16)
                                for w in (16, 8):
                                    nc.vector.tensor_tensor(
                                        v[:, :, 0:w], v[:, :, 0:w],
                                        v[:, :, w:2 * w], ops.max)
                                nc.vector.tensor_reduce(
                                    dst, v[:, :, 0:8],
                                    mybir.AxisListType.X, ops.max)
                            else:  # 'S'
                                v = ps.rearrange("p (c w) -> p c w", c=16)
                                nc.vector.tensor_reduce(
                                    dst, v, mybir.AxisListType.X, ops.max)
                            dr += 1
                            if dr % 16 == 0 or dr == nd:
                                d0 = 16 * ((dr - 1) // 16)
                                nc.gpsimd.dma_start(
                                    out_d.ap()[:, 16 * d0:16 * dr],
                                    acc[:, 0:16 * (dr - d0)])
    nc.compile()
    return nc


def kernel(heatmaps, affine_trans, cam_Intri, cam_R, cam_T, inv_affine_trans):
    from concourse.bass_utils import run_bass_kernel_spmd

    heatmaps = np.asarray(heatmaps)
    in_dtype = heatmaps.dtype

    idx, raw = _line_coords(affine_trans, cam_Intri, cam_R, cam_T,
                            inv_affine_trans)
    perms, scheds, nstrip, nitem = _schedule(idx, raw)

    hm16 = np.asarray(heatmaps, np.float32).reshape(NVIEW, C, H, W)
    hm16 = hm16.astype(np.float16)
    # zero-pad so y-windows may overhang past 63
    hmp_x = np.zeros((NVIEW, C, H + WIN, W), np.float16)
    hmp_x[:, :, :H, :] = hm16
    hmp_y = np.zeros((NVIEW, C, H, W + WIN), np.float16)
    hmp_y[:, :, :, :W] = hm16

    in_maps = []
    for ci in range(NCORE):
        msk, tab = _pack_core(scheds[ci], idx, perms, hmp_x, hmp_y, nstrip)
        in_maps.append({"msk": msk, "tab": tab})

    key = (nstrip, nitem)
    if key not in _COMPILED:
        _COMPILED[key] = _build_program(nstrip, nitem)
    nc = _COMPILED[key]

    res = run_bass_kernel_spmd(nc, in_maps, list(range(NCORE)))

    out_full = np.zeros((NPAIR, C, HW), np.float32)
    for ci in range(NCORE):
        ob = res.results[ci]["out"].astype(np.float32)
        agg = {}
        dr = 0
        for b, (gk, items) in enumerate(scheds[ci]):
            if _cal(b) == 'A':
                continue
            ev = ob[:, 16 * dr:16 * dr + 16]
            dr += 1
            if gk is None:
                continue
            if gk in agg:
                np.maximum(agg[gk], ev, out=agg[gk])
            else:
                agg[gk] = ev.copy()
        for (p, blk), cur in agg.items():
            px = perms[p][blk * PXB:(blk + 1) * PXB]
            out_full[p][:, px] = cur.T

    out = np.zeros((NVIEW, NVIEW - 1, C, H, W), np.float32)
    for p, (c, o) in enumerate(_PAIRS):
        slot = [v for v in range(NVIEW) if v != c].index(o)
        out[c, slot] = out_full[p].reshape(C, H, W)
    return out.astype(in_dtype, copy=False)


# revision 6
# speedup vs baseline: 1.0501x; 1.0000x over previous
"""Trainium2 Bass kernel for nn_CamFusionModule (epipolar max-sampling fusion).

Design (host-scheduled windowed gather):

Host (bit-exact jax-CPU camera math, as the reference):
  * per (pair, sweep, t) rounded sample indices for all 4096 pixels.
  * pixels sorted per pair by epipolar-line parameter -> 128-px blocks
    whose index values cluster into narrow y-windows.
  * work items (pair, sweep, t-pair, px-block, y-window of 16): for each,
    a one-hot fp8 mask [32=(2 parity x 16 y_off), 128 px] and an fp16
    table [32, 32=(2 parity x 16 ch)] holding the heatmap samples.
  * items are grouped by (pair, px-block), padded to 16 (= one PSUM
    bank), bin-packed across the 8 cores, and packed into DMA strips.

Device (identical SPMD program on 8 cores; only the data differs):
  stream strips -> per item one K=32 matmul (mask stationary fp8,
  table moving fp16) gathering 2x16-channel samples for 128 px into a
  PSUM bank slot; after 16 items, max-reduce the bank [128, 512] ->
  [128, 16] (DVE tensor_reduce / GPSIMD max tree, 2:1 split); batch
  results stream to DRAM.

Host combines per-group batches (max), unpermutes pixels, reassembles
[4, 3, 16, 64, 64]. Zero padding is exact: heatmaps are non-negative
and the reference floors partially-OOB lines at 0.
"""

import numpy as np
import ml_dtypes

NVIEW = 4
B, C, H, W = 1, 16, 64, 64
HW = H * W
NPAIR = 12
NCORE = 8
PXB = 128            # pixels per matmul block (M)
WIN = 16             # y-window height
BANK = 16            # items per PSUM bank / drain batch
SPF = 64             # items per strip per partition slot
SLOTS = 3            # partition slots per strip (base 0/32/64)
SITEMS = SPF * SLOTS  # 192 items per strip

_PAIRS = [(c, o) for c in range(NVIEW) for o in range(NVIEW) if o != c]
_F8 = ml_dtypes.float8_e4m3


def _line_coords(affine_trans, cam_Intri, cam_R, cam_T, inv_affine_trans):
    """Reference-exact rounded sample indices.
    Returns idx [12, 2, 64, 4096] float32 where idx[p, 0, t, px] is the
    x-sweep row index (sample hm[o, ch, idx, t]) and idx[p, 1, t, px] the
    y-sweep column index (sample hm[o, ch, t, idx]); invalid -> -1."""
    import jax
    import jax.numpy as jnp
    cpu = jax.devices("cpu")[0]
    with jax.default_device(cpu):
        V = NVIEW
        h, w = H, W
        BIG = 1.0e9
        yy, xx = jnp.meshgrid(jnp.arange(h, dtype=jnp.float32),
                              jnp.arange(w, dtype=jnp.float32), indexing='ij')
        onehm = jnp.stack([xx.reshape(-1), yy.reshape(-1),
                           jnp.ones(HW, jnp.float32)], 0)
        K = jnp.asarray(cam_Intri).reshape(B, V, 3, 3)
        R = jnp.asarray(cam_R).reshape(B, V, 3, 3)
        T = jnp.asarray(cam_T).reshape(B, V, 3, 1)
        Aff = jnp.asarray(affine_trans).reshape(B, V, 3, 3)
        invAff = jnp.asarray(inv_affine_trans).reshape(B, V, 3, 3)
        invK = jnp.linalg.inv(K)
        ray = jnp.einsum('bvij,bvjk,kp->bvip', invK, invAff, onehm)
        deps = jnp.array([1000.0, 5000.0], jnp.float32).reshape(2, 1, 1, 1, 1)
        xg = jnp.einsum('bvji,dbvjp->dbvip', R, deps * ray[None]) + T[None]
        xcam = jnp.einsum('boij,dbcojp->dbcoip', R, xg[:, :, :, None] - T[:, None])
        xnorm = xcam / xcam[:, :, :, :, 2:3]
        M = jnp.einsum('bvij,bvjk->bvik', Aff, K)
        uv = jnp.einsum('boij,dbcojp->dbcoip', M, xnorm)
        oth = np.array([[o for o in range(V) if o != c] for c in range(V)])
        uv = uv[:, :, jnp.arange(V)[:, None], oth]
        x0, y0 = uv[0, ..., 0, :], uv[0, ..., 1, :]
        x1, y1 = uv[1, ..., 0, :], uv[1, ..., 1, :]
        kk = (y1 - y0) / (x1 - x0)
        xs = jnp.arange(w, dtype=jnp.float32)
        ysw = kk[..., None] * (xs - x0[..., None]) + y0[..., None]
        ysh = jnp.arange(h, dtype=jnp.float32)
        xsh = (ysh - y0[..., None]) / kk[..., None] + x0[..., None]

        def _round_chain(v):
            v = jnp.where(jnp.isfinite(v), v, jnp.float32(BIG))
            g = v / jnp.float32((W - 1) / 2.0) - 1.0
            return jnp.round((g + 1.0) * 0.5 * (W - 1))

        iy = np.asarray(_round_chain(ysw), np.float32)  # (B,V,V-1,HW,w)
        ix = np.asarray(_round_chain(xsh), np.float32)
    iy = iy.reshape(NPAIR, HW, W).transpose(0, 2, 1)    # [12, t, px]
    ix = ix.reshape(NPAIR, HW, H).transpose(0, 2, 1)
    idx = np.stack([iy, ix], axis=1)                    # [12, 2, 64, 4096]
    raw = np.clip(idx, -3000.0, 3000.0).astype(np.float32)
    valid = (idx >= 0) & (idx <= 63)
    idx = np.where(valid, idx, -1.0).astype(np.float32)
    return idx, raw


def _schedule(idx, raw):
    """Host scheduler. Returns per-pair perms and per-core schedules.

    Each schedule is a list of batches; each batch is (group_key, items)
    with exactly BANK items (None-padded); item = (p, s, g, blk, wb)."""
    perms = np.empty((NPAIR, HW), np.int64)
    groups = {}   # (p, blk) -> list of items
    for p in range(NPAIR):
        key1 = raw[p, 0, 32]
        key2 = raw[p, 0, 48] - raw[p, 0, 16]
        perm = np.lexsort((key2, key1))
        perms[p] = perm
        for s in range(2):
            a = idx[p, s][:, perm]                      # [64, 4096]
            for blk in range(HW // PXB):
                sl = a[:, blk * PXB:(blk + 1) * PXB]
                for g in range(W // 2):
                    rows = sl[2 * g:2 * g + 2]
                    vv = rows >= 0
                    if not vv.any():
                        continue
                    vals = rows[vv]
                    lo, hi = int(vals.min()), int(vals.max())
                    for wb in range(lo, hi + 1, WIN):
                        groups.setdefault((p, blk), []).append(
                            (p, s, g, blk, wb))
    # pad each group to a multiple of BANK
    for k, items in groups.items():
        pad = (-len(items)) % BANK
        items.extend([None] * pad)
    # greedy bin-pack groups across cores
    order = sorted(groups, key=lambda k: -len(groups[k]))
    loads = [0] * NCORE
    core_groups = [[] for _ in range(NCORE)]
    for k in order:
        c = int(np.argmin(loads))
        core_groups[c].append(k)
        loads[c] += len(groups[k])

    # pack each core's banks into the drain calendar: period-7 pattern
    # [A B A B A B S]; (A,B) positions form a pair that must hold two
    # banks of the same group; S holds one bank.
    pad_bank = (None, [None] * BANK)
    core_banks = []
    for c in range(NCORE):
        rem = {}
        for k in core_groups[c]:
            items = groups[k]
            rem[k] = [items[b0:b0 + BANK]
                      for b0 in range(0, len(items), BANK)]
        seq = []
        while rem:
            t = _cal(len(seq))
            if t == 'A':
                k = max(rem, key=lambda g: len(rem[g]))
                if len(rem[k]) >= 2:
                    seq.append((k, rem[k].pop(0)))
                    seq.append((k, rem[k].pop(0)))
                else:
                    seq.append((k, rem[k].pop(0)))
                    seq.append((k, [None] * BANK))
                if not rem[k]:
                    del rem[k]
            else:  # S
                odd = [g for g in rem if len(rem[g]) % 2 == 1]
                k = min(odd, key=lambda g: len(rem[g])) if odd else \
                    max(rem, key=lambda g: len(rem[g]))
                seq.append((k, rem[k].pop(0)))
                if not rem[k]:
                    del rem[k]
        core_banks.append(seq)
    nbank = max(len(s) for s in core_banks)
    # align all cores to nbank with pad banks (calendar-safe: pads can
    # sit at any position; a pad pair or pad single drains zeros)
    while _cal(nbank) == 'B':
        nbank += 1
    for s in core_banks:
        while len(s) < nbank:
            s.append(pad_bank)
    nitem = nbank * BANK
    nstrip = -(-nitem // SITEMS)
    return perms, core_banks, nstrip, nitem


_CAL13 = ('S', 'A', 'B', 'A', 'B', 'A', 'B',
          'U', 'A', 'B', 'A', 'B', 'S')


def _cal(b):
    """Drain calendar: bank position -> 'A' (pair first), 'B' (pair
    second, drains both), 'S' (single direct drain). Singles lead each
    period so the DVE can start before the first ACT copies land; an
    extra run of singles at the very start hides the ACT warm-up."""
    if b < 3:
        return 'S'
    return _CAL13[(b + 4) % 13]


def _ndrains(nbank):
    return sum(1 for b in range(nbank) if _cal(b) in ('B', 'S', 'U'))


def _pack_core(sched, idx, perms, hmp_x, hmp_y, nstrip):
    """Build one core's strip arrays from its schedule."""
    msk = np.zeros((nstrip, 96, SPF * PXB), _F8)
    tab = np.zeros((nstrip, 96, SPF * 32), np.float16)
    yoff = np.arange(WIN, dtype=np.float32)
    it = 0
    for (gk, items) in sched:
        for item in items:
            st, loc = divmod(it, SITEMS)
            j, f = divmod(loc, SPF)
            it += 1
            if item is None:
                continue
            p, s, g, blk, wb = item
            o = _PAIRS[p][1]
            px = perms[p][blk * PXB:(blk + 1) * PXB]
            rows = idx[p, s][2 * g:2 * g + 2][:, px]      # [2, 128]
            m = (rows[:, None, :] == (wb + yoff)[None, :, None])
            msk[st, 32 * j:32 * j + 32, f * PXB:(f + 1) * PXB] = \
                m.reshape(32, PXB).astype(_F8)
            # table [32, 32]: row par*16+y, col par*16+ch (block diagonal)
            t32 = np.zeros((32, 32), np.float16)
            for par in range(2):
                t = 2 * g + par
                if s == 0:
                    blkv = hmp_x[o, :, wb:wb + WIN, t]    # [ch, y]
                else:
                    blkv = hmp_y[o, :, t, wb:wb + WIN]    # [ch, x]
                t32[par * 16:par * 16 + 16, par::2] = blkv.T
            tab[st, 32 * j:32 * j + 32, f * 32:(f + 1) * 32] = t32
    return msk, tab


_COMPILED = {}


DR_MOD = 10       # of every DR_MOD banks, DR_ASSIST drain via ACT copy
DR_ASSIST = 9


def _build_program(nstrip, nitem):
    import concourse.bacc as bacc
    import concourse.mybir as mybir
    import concourse.tile as tile
    from contextlib import ExitStack

    dt = mybir.dt
    ops = mybir.AluOpType
    nb = nitem // BANK
    nd = _ndrains(nb)

    nc = bacc.Bacc("TRN2", target_bir_lowering=False, debug=False,
                   num_devices=NCORE)
    msk_d = nc.dram_tensor("msk", [nstrip, 96, SPF * PXB], dt.float8e4,
                           kind="ExternalInput")
    tab_d = nc.dram_tensor("tab", [nstrip, 96, SPF * 32], dt.float16,
                           kind="ExternalInput")
    out_d = nc.dram_tensor("out", [128, nd * 16], dt.float16,
                           kind="ExternalOutput")

    with tile.TileContext(nc) as tc:
        with ExitStack() as ctx:
            spool = ctx.enter_context(tc.tile_pool(name="strips", bufs=7))
            ppool = ctx.enter_context(tc.tile_pool(name="banks", bufs=2,
                                                   space="PSUM"))
            ppool2 = ctx.enter_context(tc.tile_pool(name="pbanks", bufs=3,
                                                    space="PSUM"))
            apool = ctx.enter_context(tc.tile_pool(name="accs", bufs=4))
            dpool = ctx.enter_context(tc.tile_pool(name="scr", bufs=6))

            acc = None
            ps = None
            scrA = None
            dr = 0          # drain event counter
            for st in range(nstrip):
                mk = spool.tile([96, SPF * PXB], dt.float8e4, tag="mk")
                tb = spool.tile([96, SPF * 32], dt.float16, tag="tb")
                if st == 0:
                    # split first strip: unlock banks in consumption order
                    nc.sync.dma_start(mk[0:32, 0:16 * PXB],
                                      msk_d.ap()[0, 0:32, 0:16 * PXB])
                    nc.sync.dma_start(tb[0:32, 0:16 * 32],
                                      tab_d.ap()[0, 0:32, 0:16 * 32])
                    nc.sync.dma_start(mk[0:32, 16 * PXB:],
                                      msk_d.ap()[0, 0:32, 16 * PXB:])
                    nc.sync.dma_start(mk[32:96, :], msk_d.ap()[0, 32:96, :])
                    nc.sync.dma_start(tb[0:32, 16 * 32:],
                                      tab_d.ap()[0, 0:32, 16 * 32:])
                    nc.sync.dma_start(tb[32:96, :], tab_d.ap()[0, 32:96, :])
                else:
                    nc.sync.dma_start(mk[:], msk_d.ap()[st])
                    nc.sync.dma_start(tb[:], tab_d.ap()[st])
                for j in range(SLOTS):
                    for f in range(SPF):
                        k = st * SITEMS + j * SPF + f
                        if k >= nitem:
                            break
                        bs = k % BANK
                        if bs == 0:
                            bt = _cal(k // BANK)
                            if bt == 'A':
                                ps2 = ppool2.tile([128, 1024], dt.float32,
                                                  tag="pair")
                                ps = ps2[:, 0:512]
                            elif bt == 'B':
                                ps = ps2[:, 512:1024]
                            else:
                                pst = ppool.tile([128, 512], dt.float32,
                                                 tag="bank", name="sbank")
                                ps = pst[:]
                        pv = ps.rearrange("p (c s q) -> p c s q",
                                          c=16, s=16, q=2)
                        nc.tensor.matmul(
                            pv[:, :, bs, :],
                            mk[32 * j:32 * j + 32, PXB * f:PXB * (f + 1)],
                            tb[32 * j:32 * j + 32, 32 * f:32 * (f + 1)],
                            start=True, stop=True)
                        if bs == BANK - 1:
                            b = k // BANK
                            t = _cal(b)
                            if t == 'A':
                                continue
                            if dr % 16 == 0:
                                acc = apool.tile([128, 256], dt.float16,
                                                 tag="acc")
                            dst = acc[:, 16 * (dr % 16):16 * (dr % 16) + 16]
                            if t == 'B':
                                # one ACT copy evacuates both banks of
                                # the pair, then fp16 DVE max + folds
                                scr2 = dpool.tile([128, 1024], dt.float16,
                                                  tag="scr2")
                                nc.scalar.copy(scr2[:], ps2[:])
                                h0 = scr2[:, 0:512]
                                nc.vector.tensor_tensor(
                                    h0, h0, scr2[:, 512:1024], ops.max)
                                v = h0.rearrange("p (c w) -> p c w", c=16)
                                for w in (16, 8):
                                    nc.vector.tensor_tensor(
                                        v[:, :, 0:w], v[:, :, 0:w],
                                        v[:, :, w:2 * w], ops.max)
                                nc.vector.tensor_reduce(
                                    dst, v[:, :, 0:8],
                                    mybir.AxisListType.X, ops.max)
                            elif t == 'U':
                                scr = dpool.tile([128, 512], dt.float16,
                                                 tag="scrU")
                                nc.scalar.copy(scr[:], ps)
                                v = scr[:].rearrange("p (c w) -> p c w", c=16)
                                for w in (16, 8):
                                    nc.vector.tensor_tensor(
                                        v[:, :, 0:w], v[:, :, 0:w],
                                        v[:, :, w:2 * w], ops.max)
                                nc.vector.tensor_reduce(
                                    dst, v[:, :, 0:8],
                                    mybir.AxisListType.X, ops.max)
                            else:  # 'S'
                                v = ps.rearrange("p (c w) -> p c w", c=16)
                                nc.vector.tensor_reduce(
                                    dst, v, mybir.AxisListType.X, ops.max)
                            dr += 1
                            if dr % 16 == 0 or dr == nd:
                                d0 = 16 * ((dr - 1) // 16)
                                nc.gpsimd.dma_start(
                                    out_d.ap()[:, 16 * d0:16 * dr],
                                    acc[:, 0:16 * (dr - d0)])
    nc.compile()
    return nc


def kernel(heatmaps, affine_trans, cam_Intri, cam_R, cam_T, inv_affine_trans):
    from concourse.bass_utils import run_bass_kernel_spmd

    heatmaps = np.asarray(heatmaps)
    in_dtype = heatmaps.dtype

    idx, raw = _line_coords(affine_trans, cam_Intri, cam_R, cam_T,
                            inv_affine_trans)
    perms, scheds, nstrip, nitem = _schedule(idx, raw)

    hm16 = np.asarray(heatmaps, np.float32).reshape(NVIEW, C, H, W)
    hm16 = hm16.astype(np.float16)
    # zero-pad so y-windows may overhang past 63
    hmp_x = np.zeros((NVIEW, C, H + WIN, W), np.float16)
    hmp_x[:, :, :H, :] = hm16
    hmp_y = np.zeros((NVIEW, C, H, W + WIN), np.float16)
    hmp_y[:, :, :, :W] = hm16

    in_maps = []
    for ci in range(NCORE):
        msk, tab = _pack_core(scheds[ci], idx, perms, hmp_x, hmp_y, nstrip)
        in_maps.append({"msk": msk, "tab": tab})

    key = (nstrip, nitem)
    if key not in _COMPILED:
        _COMPILED[key] = _build_program(nstrip, nitem)
    nc = _COMPILED[key]

    res = run_bass_kernel_spmd(nc, in_maps, list(range(NCORE)))

    out_full = np.zeros((NPAIR, C, HW), np.float32)
    for ci in range(NCORE):
        ob = res.results[ci]["out"].astype(np.float32)
        agg = {}
        dr = 0
        for b, (gk, items) in enumerate(scheds[ci]):
            if _cal(b) == 'A':
                continue
            ev = ob[:, 16 * dr:16 * dr + 16]
            dr += 1
            if gk is None:
                continue
            if gk in agg:
                np.maximum(agg[gk], ev, out=agg[gk])
            else:
                agg[gk] = ev.copy()
        for (p, blk), cur in agg.items():
            px = perms[p][blk * PXB:(blk + 1) * PXB]
            out_full[p][:, px] = cur.T

    out = np.zeros((NVIEW, NVIEW - 1, C, H, W), np.float32)
    for p, (c, o) in enumerate(_PAIRS):
        slot = [v for v in range(NVIEW) if v != c].index(o)
        out[c, slot] = out_full[p].reshape(C, H, W)
    return out.astype(in_dtype, copy=False)
